# revision 31
# baseline (speedup 1.0000x reference)
"""Trainium2 Bass kernel for nn_DecoderLayer_43877385896448 (see spec).

Decoder layer with sigmoid linear attention (rank-1 per head), 2 attn blocks,
FFN, 3 layernorms.  B=4, S=4096, D=1024, H=16 heads (depth-1 q/k per head),
F=4096.

Sharding: rows (b, s) split across 8 cores -> core c owns batch b=c//2,
sequence half (c%2)*2048.  All matmuls are row-parallel with replicated
weights; the only cross-core exchange is an AllReduce of the tiny per-batch
attention state kv[16,65] (one per attention block, overlapped with compute).

Key simplifications vs a naive mapping:
 - attention output = sigmoid(q) @ M_aug with M_aug = blockdiag(cumsum kv)@wo
   (+bias fold), a [S,17]x[17,D] matmul instead of [S,D]x[D,D].
 - kv state = (sigmoid(k)^T x) @ wv instead of sigmoid(k)^T (x @ wv):
   a [16,D]x[D,D] matmul instead of the full [S,D]x[D,D] v-projection.
 - x^T / enc^T are packed host-side (bf16), so no on-device transposes of
   the inputs; q1,k1 are computed in one fused projection pass over x^T.
"""

import numpy as np
import ml_dtypes

import concourse.bass as bass
import concourse.bacc as bacc
import concourse.tile as tile
import concourse.mybir as mybir
from concourse import masks
from concourse.bass_utils import run_bass_kernel_spmd

F32 = mybir.dt.float32
F32R = mybir.dt.float32r
BF16 = mybir.dt.bfloat16
AF = mybir.ActivationFunctionType
ALU = mybir.AluOpType
AX = mybir.AxisListType

B, S, D, H, FF = 4, 4096, 1024, 16, 4096
DV = D // H            # 64
P = 128
N_CORES = 8
S_LOC = 2048           # rows per core
T = S_LOC // P         # 16 s-tiles per core
KT = D // P            # 8 k-tiles over D
MT = FF // P           # 32 dff tiles
EPS = 1e-6
import os
NO_CC = bool(int(os.environ.get("BASS_NO_CC", "0")))
SQ = 4                 # ffn processes s in 4 quarters of 512 rows
TQ = T // SQ
NCH = S_LOC // 512     # 4 projection chunks of 512 cols


def build_program(affine_trivial=False):
    nc = bacc.Bacc("TRN2", target_bir_lowering=False, debug=False,
                   num_devices=N_CORES)

    # ---- DRAM I/O ----
    d = {}

    def din(name, shape, dtype=F32):
        d[name] = nc.dram_tensor(name, list(shape), dtype,
                                 kind="ExternalInput").ap()

    din("x_loc", [S_LOC, D], BF16)
    din("enc_loc", [S_LOC, D], BF16)
    din("xTb", [P, KT, S_LOC], BF16)
    din("encTb", [P, KT, S_LOC], BF16)
    din("wqk1p", [P, KT, 5 * H], BF16)
    din("wk2p", [P, KT, H], BF16)
    din("wq2r", [P, KT, H], F32R)
    din("c2b", [P, H]); din("bq2b", [P, H])
    for w in ["wv1p", "wo1p", "wv2p", "wo2p"]:
        din(w, [P, KT, D], BF16)
    din("wf1p", [MT, P, KT, P], BF16)
    din("wf2p", [P, MT, D], BF16)
    din("bq1c", [H, 1]); din("bq2c", [H, 1])
    din("bk1c", [H, 1]); din("bk2c", [H, 1])
    din("bv1h", [H, DV]); din("bv2h", [H, DV])
    din("bo1r", [H, D], BF16); din("bo2r", [H, D], BF16)
    din("bf1c", [P, MT]); din("bf2b", [P, D], F32R)
    for v in ["g1b", "be1b", "g2b", "be2b", "g3b", "be3b"]:
        din(v, [P, D])
    din("maskh", [H, D]); din("maskT", [P, KT, H])
    din("U16", [H, H]); din("Bsel", [64, H]); din("BselT", [H, 64])
    out_dram = nc.dram_tensor("out_loc", [S_LOC, D], F32,
                              kind="ExternalOutput").ap()

    with tile.TileContext(nc) as tc:
        def pool(name, bufs, side="left", space="SBUF"):
            return tc.alloc_tile_pool(name=name, bufs=bufs, side=side,
                                      space=space)

        # ============ long-lived pools ============
        cpool = pool("consts", 1)
        wf1_pool = pool("wf1", 4)
        ln_pool = pool("ln", 2, side="right")
        state_pool = pool("state", 1, side="right")
        apool = pool("attnaux", 1, side="right")
        sigq2_pool = pool("sigq2", 1, side="right")
        xk_pool = pool("xkeep", 1, side="right")
        row_pool = pool("rows", 5, side="right")
        sigq1_pool = pool("sigq1", 1, side="right")
        ma1_pool = pool("ma1", 1, side="right")
        sigk_pool = pool("sigk", 1, side="right")

        ident = cpool.tile([P, P], F32, name="ident")
        masks.make_identity(nc, ident[:])
        identr = cpool.tile([P, P], F32R, name="identr")
        nc.vector.tensor_copy(identr[:], ident[:])
        identb = cpool.tile([P, P], BF16, name="identb")
        nc.vector.tensor_copy(identb[:], ident[:])

        def load_const(key, dtype=F32, pl=None):
            pl = pl if pl is not None else cpool
            t_ = pl.tile([int(s) for s in d[key].shape], dtype,
                         name=f"c_{key}")
            nc.sync.dma_start(t_[:], d[key][:])
            return t_

        wqk1 = load_const("wqk1p", BF16)
        wk2 = load_const("wk2p", BF16)
        maskh = load_const("maskh")
        maskT = load_const("maskT")
        U16 = load_const("U16")
        Bsel = load_const("Bsel")
        BselT = load_const("BselT")
        bq1c = load_const("bq1c"); bq2c = load_const("bq2c")
        bk1c = load_const("bk1c"); bk2c = load_const("bk2c")
        bv1h = load_const("bv1h"); bv2h = load_const("bv2h")
        bo1r = load_const("bo1r", BF16); bo2r = load_const("bo2r", BF16)
        bf1c = load_const("bf1c")
        bf2br = load_const("bf2b", F32R)
        wq2r = load_const("wq2r", F32R)
        c2b = load_const("c2b"); bq2b = load_const("bq2b")
        eps = cpool.tile([P, 1], F32, name="epsc")
        nc.vector.memset(eps[:], EPS)

        dram_pool = pool("ccdram", 1, space="DRAM")
        cc1_in = dram_pool.tile([64, 65], F32, name="cc1_in")
        cc1_out = dram_pool.tile([64, 65], F32, name="cc1_out")
        cc2_in = dram_pool.tile([64, 65], F32, name="cc2_in")
        cc2_out = dram_pool.tile([64, 65], F32, name="cc2_out")
        out1d = dram_pool.tile([S_LOC, D], BF16, name="out1d")
        out2d = dram_pool.tile([S_LOC, D], BF16, name="out2d")

        sigq1 = sigq1_pool.tile([H, S_LOC], F32R, name="sigq1")
        sigq2 = sigq2_pool.tile([H, S_LOC], F32R, name="sigq2")
        sigk1 = sigk_pool.tile([P, T, H], BF16, name="sigk1")
        sigk2 = sigk_pool.tile([P, T, H], BF16, name="sigk2")

        def layernorm(res_ps, out_slice, g_sb, be_sb, lnp, tname,
                      rstd_out=None, nmr_out=None):
            """LN of psum tile res_ps [128,1024] -> out_slice (SBUF)."""
            st6 = lnp.tile([P, 2, 6], F32, tag="st6", name=f"st6_{tname}")
            nc.vector.bn_stats(st6[:, 0, :], res_ps[:, 0:512])
            nc.vector.bn_stats(st6[:, 1, :], res_ps[:, 512:1024])
            mv = lnp.tile([P, 2], F32, tag="mv", name=f"mv_{tname}")
            nc.vector.bn_aggr(mv[:], st6[:])
            std = lnp.tile([P, 1], F32, tag="std", name=f"std_{tname}")
            nc.scalar.activation(std[:], mv[:, 1:2], AF.Sqrt, bias=eps[:])
            if rstd_out is None:
                rstd = lnp.tile([P, 1], F32, tag="rstd", name=f"rstd_{tname}")
                rstd = rstd[:]
            else:
                rstd = rstd_out
            nc.vector.reciprocal(rstd, std[:])
            if nmr_out is None:
                nmr = lnp.tile([P, 1], F32, tag="nmr", name=f"nmr_{tname}")
                nmr = nmr[:]
            else:
                nmr = nmr_out
            nc.vector.scalar_tensor_tensor(nmr, mv[:, 0:1], -1.0, rstd,
                                           op0=ALU.mult, op1=ALU.mult)
            if affine_trivial:
                nc.scalar.activation(out_slice, res_ps[:], AF.Identity,
                                     bias=nmr, scale=rstd)
            else:
                xh = lnp.tile([P, 1024], F32, tag="ot", name=f"xh_{tname}")
                nc.scalar.activation(xh[:], res_ps[:], AF.Identity,
                                     bias=nmr, scale=rstd)
                nc.vector.scalar_tensor_tensor(out_slice, xh[:], 1.0, g_sb[:],
                                               op0=ALU.mult, op1=ALU.mult)
                nc.vector.tensor_add(out_slice, out_slice, be_sb[:])
            return rstd, nmr

        def attn_state(G_ps, sksum, tp2_psum, cc_in, nm):
            """G psum [16,1024] + sigma(k)-rowsum [16,1] -> kv[16,65]
            -> batch-slot select -> DMA."""
            gtmp = state_pool.tile([16, 1024], F32, tag="gtmp",
                                   name=f"gtmp_{nm}")
            nc.vector.tensor_mul(gtmp[:], G_ps[:, 0:1024], maskh[:])
            kvp = state_pool.tile([16, 65], F32, tag="kvp", name=f"kvp_{nm}")
            nc.vector.tensor_reduce(
                kvp[:, 0:64], gtmp[:].rearrange("p (c v) -> p v c", v=DV),
                axis=AX.X, op=ALU.add)
            nc.vector.tensor_copy(kvp[:, 64:65], sksum[:])
            kvsel_ps = tp2_psum.tile([64, 65], F32, tag="sel",
                                     name=f"kvselp_{nm}")
            nc.tensor.matmul(kvsel_ps[:], BselT[:], kvp[:],
                             start=True, stop=True)
            kvsel = state_pool.tile([64, 65], F32, tag="kvsel",
                                    name=f"kvsel_{nm}")
            nc.vector.tensor_copy(kvsel[:], kvsel_ps[:])
            nc.sync.dma_start(cc_in[:], kvsel[:])

        def state_to_M(cc_out, wo_sb, bvh, bor, spsum, ma_pool, nm):
            """AllReduce out -> own-batch kv -> cumsum -> M_aug [16,1024]."""
            kvred = state_pool.tile([64, 65], F32, tag="kvred",
                                    name=f"kvred_{nm}")
            nc.sync.dma_start(kvred[:], cc_out[:])
            kvmy_ps = spsum.tile([16, 65], F32, tag="sm", name=f"kvmyp_{nm}")
            nc.tensor.matmul(kvmy_ps[:], Bsel[:], kvred[:],
                             start=True, stop=True)
            kvmy = state_pool.tile([16, 65], F32, tag="kvmy",
                                   name=f"kvmy_{nm}")
            nc.vector.tensor_copy(kvmy[:], kvmy_ps[:])
            kv_bv = state_pool.tile([16, 64], F32, tag="kv_bv",
                                    name=f"kv_bv_{nm}")
            nc.vector.scalar_tensor_tensor(
                kv_bv[:], bvh[:], kvmy[:, 64:65], kvmy[:, 0:64],
                op0=ALU.mult, op1=ALU.add)
            scum_ps = spsum.tile([16, 64], F32, tag="sm", name=f"scump_{nm}")
            nc.tensor.matmul(scum_ps[:], U16[:], kv_bv[:],
                             start=True, stop=True)
            scum = state_pool.tile([16, 64], F32, tag="scumsb",
                                   name=f"scum_{nm}")
            nc.vector.tensor_copy(scum[:], scum_ps[:])
            scumT_ps = spsum.tile([64, 16], F32, tag="sm", name=f"scumTp_{nm}")
            nc.tensor.matmul(scumT_ps[:], scum[:], ident[:16, :16],
                             is_transpose=True)
            scumT2 = state_pool.tile([P, 16], F32, tag="scumT2",
                                     name=f"scumT2_{nm}")
            nc.vector.tensor_copy(scumT2[0:64, :], scumT_ps[:])
            nc.vector.tensor_copy(scumT2[64:P, :], scumT_ps[:])
            bdts = []
            for kt in range(KT):
                bdt = state_pool.tile([P, 16], BF16, tag=f"bdt{kt}",
                                      name=f"bdt_{nm}_{kt}")
                nc.vector.tensor_mul(bdt[:], scumT2[:], maskT[:, kt, :])
                bdts.append(bdt)
            Ma = ma_pool.tile([H, 1024], F32R, name=f"Ma_{nm}")
            for n2 in range(2):
                sl = slice(n2 * 512, (n2 + 1) * 512)
                M_ps = spsum.tile([16, 512], F32, tag="sm",
                                  name=f"M_{nm}_{n2}")
                for kt in range(KT):
                    nc.tensor.matmul(M_ps[:], bdts[kt][:], wo_sb[:, kt, sl],
                                     start=(kt == 0), stop=False)
                nc.tensor.matmul(M_ps[:], identb[:16, :16], bor[:, sl],
                                 start=False, stop=True)
                nc.vector.tensor_copy(Ma[:, sl], M_ps[:])
            return Ma

        def attn_side(xT_ap, x_row_ap, wproj, nheads_out, bqc, bkc, sigq_dst,
                      xT_pool_l, sigk_dst, wv_sb, cc_in, qk_psum, tp_psum,
                      ag_psum, nm, xq2_dst=None, xtc_eng=None, xk_keep=None):
            """One attention input side: projections from xT (bf16, DMA'd on
            the fly), sigmoids, A = sig(k)^T x, G = A @ wv, state -> cc."""
            sigkT = apool.tile([H, S_LOC], BF16, tag="sigkT",
                               name=f"sigkT_{nm}")
            xrows = []
            for t in range(T):
                if xk_keep is not None:
                    xr = xk_keep[:, t, :]
                else:
                    xrt = row_pool.tile([P, D], BF16, tag="xr",
                                        name=f"xr_{nm}{t}")
                    xr = xrt[:]
                nc.sync.dma_start(xr, x_row_ap[t * P:(t + 1) * P, :])
                xrows.append(xr)
            A_ps = ag_psum.tile([16, 1024], F32, tag="ag", name=f"A_{nm}")
            for n in range(NCH):
                csl = slice(n * 512, (n + 1) * 512)
                xTc = xT_pool_l.tile([P, KT, 512], BF16, tag="xTc",
                                     name=f"xTc_{nm}{n}")
                xtc_eng.dma_start(xTc[:], xT_ap[:, :, csl])
                qk_ps = qk_psum.tile([nheads_out, 512], F32, tag="qk",
                                     name=f"qk_{nm}{n}")
                for kt in range(KT):
                    nc.tensor.matmul(qk_ps[:], wproj[:, kt, :],
                                     xTc[:, kt, :],
                                     start=(kt == 0), stop=(kt == KT - 1))
                if nheads_out >= 4 * H:
                    nc.scalar.activation(sigq_dst[0:H, csl], qk_ps[0:H, :],
                                         AF.Sigmoid, bias=bqc[:])
                    nc.scalar.activation(sigkT[:, csl], qk_ps[2 * H:3 * H, :],
                                         AF.Sigmoid, bias=bkc[:])
                    if xq2_dst is not None:
                        nc.scalar.copy(xq2_dst[:, csl],
                                       qk_ps[4 * H:5 * H, :])
                else:
                    nc.scalar.activation(sigkT[:, csl], qk_ps[:], AF.Sigmoid,
                                         bias=bkc[:])
            # transpose sig(k) -> [s,16] tiles; A-matmuls (PE stays busy on
            # later qk chunks while ACT computes the earlier sigmoids)
            for t in range(T):
                skT_ps = tp_psum.tile([P, H], BF16, tag="tp",
                                      name=f"skT_{nm}{t}")
                nc.tensor.matmul(
                    skT_ps[:],
                    sigkT[:, t * P:(t + 1) * P],
                    identb[:16, :16], is_transpose=True)
                nc.vector.tensor_copy(sigk_dst[:, t, :], skT_ps[:])
                nc.tensor.matmul(A_ps[:, 0:512], sigk_dst[:, t, :],
                                 xrows[t][:, 0:512],
                                 start=(t == 0), stop=(t == T - 1))
                nc.tensor.matmul(A_ps[:, 512:1024], sigk_dst[:, t, :],
                                 xrows[t][:, 512:1024],
                                 start=(t == 0), stop=(t == T - 1))
            sksum = apool.tile([H, 1], F32, tag="sksum", name=f"sksum_{nm}")
            nc.vector.tensor_reduce(sksum[:], sigkT[:], axis=AX.X, op=ALU.add)
            A_sb = apool.tile([H, 1024], BF16, tag="A_sb", name=f"Asb_{nm}")
            nc.scalar.copy(A_sb[:], A_ps[:])
            AT = apool.tile([P, KT, H], BF16, tag="AT", name=f"AT_{nm}")
            for kt in range(KT):
                at_ps = tp_psum.tile([P, H], BF16, tag="tp",
                                     name=f"at_{nm}{kt}")
                nc.tensor.matmul(at_ps[:],
                                 A_sb[:, kt * P:(kt + 1) * P],
                                 identb[:16, :16], is_transpose=True)
                nc.vector.tensor_copy(AT[:, kt, :], at_ps[:])
            G_ps = ag_psum.tile([16, 1024], F32, tag="ag", name=f"G_{nm}")
            for kt in range(KT):
                for n2 in range(2):
                    sl = slice(n2 * 512, (n2 + 1) * 512)
                    nc.tensor.matmul(G_ps[:, sl], AT[:, kt, :],
                                     wv_sb[:, kt, sl],
                                     start=(kt == 0), stop=(kt == KT - 1))
            attn_state(G_ps, sksum, ag_psum, cc_in, nm)

        # ================= PHASE A1: x side =================
        xTc_pool = pool("xTc", 3)
        xk = xk_pool.tile([P, T, D], BF16, name="xk")
        wv1_pool = pool("wv1", 1)
        wv1 = wv1_pool.tile([P, KT, D], BF16, name="wv1")
        for kt in range(KT):
            nc.scalar.dma_start(wv1[:, kt, :], d["wv1p"][:, kt, :])

        qk_psum = pool("qk_psum", 2, space="PSUM")
        tp_psum = pool("tp_psum", 2, space="PSUM")
        ag_psum = pool("ag_psum", 1, space="PSUM")

        xq2T = sigq1_pool.tile([H, S_LOC], F32R, name="xq2T")
        attn_side(d["xTb"], d["x_loc"], wqk1, 5 * H, bq1c, bk1c, sigq1,
                  xTc_pool, sigk1, wv1, cc1_in, qk_psum, tp_psum, ag_psum,
                  "x", xq2_dst=xq2T, xtc_eng=nc.gpsimd, xk_keep=xk)
        if NO_CC:
            nc.sync.dma_start(cc1_out[:], cc1_in[:])
        else:
            nc.gpsimd.collective_compute(
                "AllReduce", ALU.add, replica_groups=[list(range(N_CORES))],
                ins=[cc1_in.opt()], outs=[cc1_out.opt()])
        wv1_pool.release()

        # ================= PHASE A2: enc side (overlaps AllReduce 1) ====
        wo1_pool = pool("wo1", 1)
        wo1 = wo1_pool.tile([P, KT, D], BF16, name="wo1")
        for kt in range(KT):
            nc.gpsimd.dma_start(wo1[:, kt, :], d["wo1p"][:, kt, :])
        wv2_pool = pool("wv2", 1)
        wv2 = wv2_pool.tile([P, KT, D], BF16, name="wv2")
        for kt in range(KT):
            nc.gpsimd.dma_start(wv2[:, kt, :], d["wv2p"][:, kt, :])

        attn_side(d["encTb"], d["enc_loc"], wk2, H, None, bk2c, None,
                  xTc_pool, sigk2, wv2, cc2_in, qk_psum, tp_psum, ag_psum,
                  "e", xtc_eng=nc.scalar)
        if NO_CC:
            nc.sync.dma_start(cc2_out[:], cc2_in[:])
        else:
            nc.gpsimd.collective_compute(
                "AllReduce", ALU.add, replica_groups=[list(range(N_CORES))],
                ins=[cc2_in.opt()], outs=[cc2_out.opt()])
        wv2_pool.release()

        # m1 state chain (waits AR1; AR1 landed during enc side)
        sps1 = pool("sps1", 1, space="PSUM")
        Ma1 = state_to_M(cc1_out, wo1, bv1h, bo1r, sps1, ma1_pool, "m1")
        sps1.release()
        wo1_pool.release()
        xTc_pool.release()
        ag_psum.release()
        tp_psum.release()
        qk_psum.release()
        sigk_pool.release()

        # ============ PHASE B: attn1 + LN1 + q2 (LN algebra) ============
        # q2 = sigmoid(rstd1*(P1^T sigq1 + (x@wq2eff)^T) + nmr1*c2 + cadd)
        # with P1 = Ma1 @ wq2eff; avoids materializing out1^T entirely.
        o2row_pool = pool("o2row", 4)
        o2T_pool = pool("o2T", 2)
        hT_pool = pool("hT", 1)
        q2aux_pool = pool("q2aux", 2)
        sq2all_pool = pool("sq2all", 1)
        sq2all = sq2all_pool.tile([P, T, H], F32, name="sq2all")
        if affine_trivial:
            gbe1_pool = g1b = be1b = None
        else:
            gbe1_pool = pool("gbe1", 1)
            g1b = load_const("g1b", pl=gbe1_pool)
            be1b = load_const("be1b", pl=gbe1_pool)

        a_psum = pool("a_psum", 2, side="right", space="PSUM")
        tpB = pool("tpB", 1, space="PSUM")
        q2ps = pool("q2ps", 1, space="PSUM")

        Ma1T = q2aux_pool.tile([P, KT, H], F32R, name="Ma1T")
        for kt in range(KT):
            mt_ps = tpB.tile([P, H], F32, tag="tp", name=f"ma1T_{kt}")
            nc.tensor.matmul(mt_ps[:],
                             Ma1[:, kt * P:(kt + 1) * P].bitcast(F32),
                             ident[:16, :16], is_transpose=True)
            nc.vector.tensor_copy(Ma1T[:, kt, :], mt_ps[:])
        P1_ps = q2ps.tile([H, H], F32, tag="q2", name="P1ps")
        for kt in range(KT):
            nc.tensor.matmul(P1_ps[:], Ma1T[:, kt, :], wq2r[:, kt, :],
                             start=(kt == 0), stop=(kt == KT - 1))
        P1 = q2aux_pool.tile([H, H], F32R, name="P1")
        nc.scalar.copy(P1[:], P1_ps[:])

        yq2sb = q2aux_pool.tile([H, S_LOC], F32R, name="yq2sb")
        rstdall = q2aux_pool.tile([P, T], F32, name="rstdall")
        nmrall = q2aux_pool.tile([P, T], F32, name="nmrall")
        for n in range(NCH):
            csl = slice(n * 512, (n + 1) * 512)
            yq2_ps = q2ps.tile([H, 512], F32, tag="q2", name=f"yq2_{n}")
            nc.tensor.matmul(yq2_ps[:], P1[:], sigq1[:, csl],
                             start=True, stop=False)
            nc.tensor.matmul(yq2_ps[:], identr[:16, :16], xq2T[:, csl],
                             start=False, stop=True)
            nc.scalar.copy(yq2sb[:, csl], yq2_ps[:])
            for t4 in range(4):
                t = n * 4 + t4
                ap_ = a_psum.tile([P, D], F32, tag="a", name=f"a1_{t}")
                for n2 in range(2):
                    sl = slice(n2 * 512, (n2 + 1) * 512)
                    nc.tensor.matmul(ap_[:, sl],
                                     sigq1[:, t * P:(t + 1) * P],
                                     Ma1[:, sl], start=True, stop=False)
                    nc.tensor.matmul(ap_[:, sl], identb[:], xk[:, t, sl],
                                     start=False, stop=True)
                o1t = ln_pool.tile([P, D], BF16, tag="otb", name=f"o1t_{t}")
                layernorm(ap_, o1t[:], g1b, be1b, ln_pool, f"ln1_{t}",
                          rstd_out=rstdall[:, t:t + 1],
                          nmr_out=nmrall[:, t:t + 1])
                nc.gpsimd.dma_start(out1d[t * P:(t + 1) * P, :], o1t[:])
        # q2 assembly tail: gate the cluster on the last LN1 so the ACT
        # sigmoid table loads once (zq2b = bq2b + 0*rstd[T-1])
        zg = q2aux_pool.tile([P, H], F32, name="zgate")
        nc.vector.memset(zg[:], 0.0)
        nc.vector.scalar_tensor_tensor(zg[:], zg[:], rstdall[:, T - 1:T],
                                       zg[:], op0=ALU.mult, op1=ALU.add)
        bq2bz = q2aux_pool.tile([P, H], F32, name="bq2bz")
        nc.vector.tensor_add(bq2bz[:], bq2b[:], zg[:])
        for t in range(T):
            yq2r_ps = tpB.tile([P, H], F32, tag="tp", name=f"yq2r_{t}")
            nc.tensor.matmul(
                yq2r_ps[:],
                yq2sb[:, t * P:(t + 1) * P].bitcast(F32),
                ident[:16, :16], is_transpose=True)
            s2 = q2aux_pool.tile([P, H], F32, tag="s2", name=f"s2_{t}")
            nc.vector.scalar_tensor_tensor(s2[:], c2b[:],
                                           nmrall[:, t:t + 1], bq2bz[:],
                                           op0=ALU.mult, op1=ALU.add)
            s1 = q2aux_pool.tile([P, H], F32, tag="s1", name=f"s1_{t}")
            nc.vector.scalar_tensor_tensor(s1[:], yq2r_ps[:],
                                           rstdall[:, t:t + 1], s2[:],
                                           op0=ALU.mult, op1=ALU.add)
            nc.scalar.activation(sq2all[:, t, :], s1[:], AF.Sigmoid)
            sq2T_ps = tpB.tile([H, P], F32, tag="tp2", name=f"sq2T_{t}")
            nc.tensor.matmul(sq2T_ps[:], sq2all[:, t, :], ident[:],
                             is_transpose=True)
            nc.vector.tensor_copy(sigq2[:, t * P:(t + 1) * P], sq2T_ps[:])
        q2ps.release()
        tpB.release()
        ma1_pool.release()
        sigq1_pool.release()
        row_pool.release()
        xk_pool.release()
        if gbe1_pool is not None:
            gbe1_pool.release()
        sq2all_pool.release()
        q2aux_pool.release()

        # ---- m2 state chain ----
        ma2_pool = pool("ma2", 1)
        if affine_trivial:
            gbe2_pool = g2b = be2b = None
        else:
            gbe2_pool = pool("gbe2", 1)
            g2b = load_const("g2b", pl=gbe2_pool)
            be2b = load_const("be2b", pl=gbe2_pool)
        o1row_pool = pool("o1row", 2)
        wo2_pool = pool("wo2", 1)
        wo2 = wo2_pool.tile([P, KT, D], BF16, name="wo2")
        for kt in range(KT):
            nc.sync.dma_start(wo2[:, kt, :], d["wo2p"][:, kt, :])
        sps2 = pool("sps2", 1, space="PSUM")
        Ma2 = state_to_M(cc2_out, wo2, bv2h, bo2r, sps2, ma2_pool, "m2")
        sps2.release()
        wo2_pool.release()

        wf2_pool = pool("wf2", 1)
        wf2 = wf2_pool.tile([P, MT, D], BF16, name="wf2")
        for m in range(MT):
            nc.gpsimd.dma_start(wf2[:, m, :], d["wf2p"][:, m, :])
        if affine_trivial:
            gbe3_pool = g3b = be3b = None
        else:
            gbe3_pool = pool("gbe3", 1)
            g3b = load_const("g3b", pl=gbe3_pool)
            be3b = load_const("be3b", pl=gbe3_pool)
        ln3_pool = pool("ln3", 2)
        h_psum = pool("h_psum", 3, space="PSUM")

        # ====== PHASE C+D: attn2+LN2 interleaved with FFN blocks ======
        def attn2_group(g):
            o2T = o2T_pool.tile([P, KT, TQ * P], BF16, tag="o2T",
                                name=f"o2T_{g}")
            for t4 in range(TQ):
                t = g * TQ + t4
                ap2 = a_psum.tile([P, D], F32, tag="a", name=f"a2_{t}")
                o1r = o1row_pool.tile([P, D], BF16, tag="o1r",
                                      name=f"o1r_{t}")
                nc.sync.dma_start(o1r[:], out1d[t * P:(t + 1) * P, :])
                for n2 in range(2):
                    sl = slice(n2 * 512, (n2 + 1) * 512)
                    nc.tensor.matmul(ap2[:, sl],
                                     sigq2[:, t * P:(t + 1) * P],
                                     Ma2[:, sl], start=True, stop=False)
                    nc.tensor.matmul(ap2[:, sl], identb[:], o1r[:, sl],
                                     start=False, stop=True)
                o2t = ln_pool.tile([P, D], BF16, tag="otb", name=f"o2t_{t}")
                layernorm(ap2, o2t[:], g2b, be2b, ln_pool, f"ln2_{t}")
                nc.gpsimd.dma_start(out2d[t * P:(t + 1) * P, :], o2t[:])
            for t4 in range(TQ):
                t = g * TQ + t4
                nc.sync.dma_start_transpose(
                    o2T[:, :, t4 * P:(t4 + 1) * P],
                    out2d[t * P:(t + 1) * P, :])
            return o2T

        def ffn_block(sq, o2T):
            """FFN for s-quarter sq; o2T arrives DMA-transposed."""
            o2rows = []
            for t4 in range(TQ):
                t = sq * TQ + t4
                o2r = o2row_pool.tile([P, D], BF16, tag="o2r",
                                      name=f"o2r_{t}")
                nc.sync.dma_start(o2r[:], out2d[t * P:(t + 1) * P, :])
                o2rows.append(o2r)
            hT = hT_pool.tile([P, MT, TQ * P], BF16, tag="hT",
                              name=f"hT_{sq}")
            for m in range(MT):
                wf1m = wf1_pool.tile([P, KT, P], BF16, tag="wf1m",
                                     name=f"wf1_{sq}_{m}")
                nc.gpsimd.dma_start(wf1m[:], d["wf1p"][m])
                hp = h_psum.tile([P, TQ * P], F32, tag="hp",
                                 name=f"hp_{sq}_{m}")
                for kt in range(KT):
                    nc.tensor.matmul(hp[:], wf1m[:, kt, :], o2T[:, kt, :],
                                     start=(kt == 0), stop=(kt == KT - 1))
                nc.scalar.activation(hT[:, m, :], hp[:], AF.Relu,
                                     bias=bf1c[:, m:m + 1])
            for t4 in range(TQ):
                t = sq * TQ + t4
                o3 = ln3_pool.tile([P, D], F32, tag="o3f", name=f"o3f_{t}")
                st6 = ln3_pool.tile([P, 2, 6], F32, tag="st6",
                                    name=f"st6f_{t}")
                chunks = []
                for n2 in range(2):
                    sl = slice(n2 * 512, (n2 + 1) * 512)
                    op3 = h_psum.tile([P, 512], F32, tag="hp",
                                      name=f"o3c_{t}_{n2}")
                    for m in range(MT):
                        nc.tensor.matmul(op3[:],
                                         hT[:, m, t4 * P:(t4 + 1) * P],
                                         wf2[:, m, sl],
                                         start=(m == 0), stop=False)
                    nc.tensor.matmul(op3[:], identb[:], o2rows[t4][:, sl],
                                     start=False, stop=affine_trivial)
                    if not affine_trivial:
                        nc.tensor.matmul(op3[:], identr[:], bf2br[:, sl],
                                         start=False, stop=True)
                    nc.vector.bn_stats(st6[:, n2, :], op3[:])
                    chunks.append(op3)
                mv = ln3_pool.tile([P, 2], F32, tag="mv", name=f"mvf_{t}")
                nc.vector.bn_aggr(mv[:], st6[:])
                std = ln3_pool.tile([P, 1], F32, tag="std", name=f"stdf_{t}")
                nc.scalar.activation(std[:], mv[:, 1:2], AF.Sqrt,
                                     bias=eps[:])
                rstd = ln3_pool.tile([P, 1], F32, tag="rstd",
                                     name=f"rstdf_{t}")
                nc.vector.reciprocal(rstd[:], std[:])
                nmr = ln3_pool.tile([P, 1], F32, tag="nmr", name=f"nmrf_{t}")
                nc.vector.scalar_tensor_tensor(nmr[:], mv[:, 0:1], -1.0,
                                               rstd[:], op0=ALU.mult,
                                               op1=ALU.mult)
                for n2 in range(2):
                    sl = slice(n2 * 512, (n2 + 1) * 512)
                    nc.scalar.activation(o3[:, sl], chunks[n2][:],
                                         AF.Identity, bias=nmr[:],
                                         scale=rstd[:])
                if not affine_trivial:
                    nc.vector.scalar_tensor_tensor(o3[:], o3[:], 1.0,
                                                   g3b[:], op0=ALU.mult,
                                                   op1=ALU.mult)
                    nc.vector.tensor_add(o3[:], o3[:], be3b[:])
                nc.scalar.dma_start(out_dram[t * P:(t + 1) * P, :], o3[:])

        o2Ts = {}
        for g in range(SQ):
            o2Ts[g] = attn2_group(g)
            if g >= 1:
                ffn_block(g - 1, o2Ts.pop(g - 1))
        ffn_block(SQ - 1, o2Ts.pop(SQ - 1))

        a_psum.release()
        rel = [h_psum, ln3_pool]
        if gbe3_pool is not None:
            rel.append(gbe3_pool)
        rel.append(wf2_pool)
        rel.append(o1row_pool)
        if gbe2_pool is not None:
            rel.append(gbe2_pool)
        rel += [ma2_pool, hT_pool, o2T_pool, o2row_pool, wf1_pool,
                cpool, dram_pool, sigq2_pool, apool, state_pool, ln_pool]
        for p_ in rel:
            p_.release()

    nc.compile()
    return nc


_NC_CACHE = {}


def _get_nc(affine_trivial):
    if affine_trivial not in _NC_CACHE:
        _NC_CACHE[affine_trivial] = build_program(affine_trivial)
    return _NC_CACHE[affine_trivial]


def _affine_trivial(inputs):
    for g in ("g1", "g2", "g3"):
        if not np.all(np.asarray(inputs[g]) == 1.0):
            return False
    for b in ("be1", "be2", "be3", "bf2"):
        if not np.all(np.asarray(inputs[b]) == 0.0):
            return False
    return True


def _prep_inputs(inputs):
    f32 = lambda a: np.ascontiguousarray(np.asarray(a, dtype=np.float32))
    x = f32(inputs["x"])
    enc = f32(inputs["enc"])

    def pack_w(w):  # [D, n] -> [P, KT, n]
        w = f32(w)
        return np.ascontiguousarray(w.reshape(KT, P, -1).transpose(1, 0, 2))

    zpad = np.zeros((D, H), np.float32)
    wq2eff = f32(inputs["g1"]).reshape(D, 1) * f32(inputs["wq2"])
    wqk1 = np.concatenate([f32(inputs["wq1"]), zpad, f32(inputs["wk1"]),
                           zpad, wq2eff], axis=1)
    cadd = (f32(inputs["be1"]) @ f32(inputs["wq2"])
            + f32(inputs["bq2"]).reshape(-1))
    shared = {
        "wqk1p": pack_w(wqk1).astype(ml_dtypes.bfloat16),
        "wk2p": pack_w(inputs["wk2"]).astype(ml_dtypes.bfloat16),
        "wq2r": pack_w(wq2eff),
        "wv1p": pack_w(inputs["wv1"]).astype(ml_dtypes.bfloat16),
        "wo1p": pack_w(inputs["wo1"]).astype(ml_dtypes.bfloat16),
        "wv2p": pack_w(inputs["wv2"]).astype(ml_dtypes.bfloat16),
        "wo2p": pack_w(inputs["wo2"]).astype(ml_dtypes.bfloat16),
    }
    shared["c2b"] = np.ascontiguousarray(
        np.broadcast_to(wq2eff.sum(0)[None, :], (P, H)))
    shared["bq2b"] = np.ascontiguousarray(
        np.broadcast_to(cadd[None, :], (P, H)))
    wf1 = f32(inputs["wf1"])  # [D, FF]
    wf1p = wf1.reshape(KT, P, MT, P).transpose(2, 1, 0, 3)
    shared["wf1p"] = np.ascontiguousarray(wf1p.astype(ml_dtypes.bfloat16))
    wf2 = f32(inputs["wf2"])  # [FF, D]
    shared["wf2p"] = np.ascontiguousarray(
        wf2.reshape(MT, P, D).transpose(1, 0, 2).astype(ml_dtypes.bfloat16))

    def bcast(v):
        v = f32(v).reshape(-1)
        return np.ascontiguousarray(np.broadcast_to(v[None, :], (P, v.size)))

    shared["bq1c"] = f32(inputs["bq1"]).reshape(H, 1)
    shared["bq2c"] = f32(inputs["bq2"]).reshape(H, 1)
    shared["bk1c"] = f32(inputs["bk1"]).reshape(H, 1)
    shared["bk2c"] = f32(inputs["bk2"]).reshape(H, 1)
    shared["bv1h"] = f32(inputs["bv1"]).reshape(H, DV)
    shared["bv2h"] = f32(inputs["bv2"]).reshape(H, DV)
    shared["bo1r"] = np.ascontiguousarray(np.broadcast_to(
        f32(inputs["bo1"])[None, :], (H, D))).astype(ml_dtypes.bfloat16)
    shared["bo2r"] = np.ascontiguousarray(np.broadcast_to(
        f32(inputs["bo2"])[None, :], (H, D))).astype(ml_dtypes.bfloat16)
    shared["bf1c"] = np.ascontiguousarray(f32(inputs["bf1"]).reshape(MT, P).T)
    shared["bf2b"] = bcast(inputs["bf2"])
    for k_src, k_dst in [("g1", "g1b"), ("be1", "be1b"), ("g2", "g2b"),
                         ("be2", "be2b"), ("g3", "g3b"), ("be3", "be3b")]:
        shared[k_dst] = bcast(inputs[k_src])

    hh = np.arange(H)
    jj = np.arange(D)
    shared["maskh"] = (jj[None, :] // DV == hh[:, None]).astype(np.float32)
    pp = np.arange(P)
    kk = np.arange(KT)
    shared["maskT"] = ((kk[None, :, None] * P + pp[:, None, None]) // DV
                       == hh[None, None, :]).astype(np.float32)
    shared["U16"] = (hh[:, None] <= hh[None, :]).astype(np.float32)

    def packT(rows):  # [S_LOC, D] f32 -> [P, KT, S_LOC] bf16
        xt = rows.T.reshape(KT, P, S_LOC).transpose(1, 0, 2)
        return np.ascontiguousarray(xt.astype(ml_dtypes.bfloat16))

    in_maps = []
    for c in range(N_CORES):
        b, half = c // 2, c % 2
        s0 = half * S_LOC
        m = dict(shared)
        xl = np.ascontiguousarray(x[b, s0:s0 + S_LOC, :])
        el = np.ascontiguousarray(enc[b, s0:s0 + S_LOC, :])
        m["x_loc"] = xl.astype(ml_dtypes.bfloat16)
        m["enc_loc"] = el.astype(ml_dtypes.bfloat16)
        m["xTb"] = packT(xl)
        m["encTb"] = packT(el)
        bsel = (np.arange(64)[:, None] == 16 * b + hh[None, :]).astype(
            np.float32)
        m["Bsel"] = bsel
        m["BselT"] = np.ascontiguousarray(bsel.T)
        in_maps.append(m)
    return in_maps


def run_on_hw(inputs, **kwargs):
    nc = _get_nc(_affine_trivial(inputs))
    in_maps = _prep_inputs(inputs)
    return run_bass_kernel_spmd(nc, in_maps, list(range(N_CORES)), **kwargs)


def kernel(**inputs):
    r = run_on_hw(inputs)
    out = np.empty((B, S, D), dtype=np.float32)
    for c in range(N_CORES):
        b, half = c // 2, c % 2
        out[b, half * S_LOC:(half + 1) * S_LOC, :] = r.results[c]["out_loc"]
    return (out, np.zeros_like(out), np.zeros_like(out))


# revision 34
# speedup vs baseline: 1.0806x; 1.0806x over previous
"""Trainium2 Bass kernel for nn_DecoderLayer_43877385896448 (see spec).

Decoder layer with sigmoid linear attention (rank-1 per head), 2 attn blocks,
FFN, 3 layernorms.  B=4, S=4096, D=1024, H=16 heads (depth-1 q/k per head),
F=4096.

Sharding: rows (b, s) split across 8 cores -> core c owns batch b=c//2,
sequence half (c%2)*2048.  All matmuls are row-parallel with replicated
weights; the only cross-core exchange is an AllReduce of the tiny per-batch
attention state kv[16,65] (one per attention block, overlapped with compute).

Key simplifications vs a naive mapping:
 - attention output = sigmoid(q) @ M_aug with M_aug = blockdiag(cumsum kv)@wo
   (+bias fold), a [S,17]x[17,D] matmul instead of [S,D]x[D,D].
 - kv state = (sigmoid(k)^T x) @ wv instead of sigmoid(k)^T (x @ wv):
   a [16,D]x[D,D] matmul instead of the full [S,D]x[D,D] v-projection.
 - x^T / enc^T are packed host-side (bf16), so no on-device transposes of
   the inputs; q1,k1 are computed in one fused projection pass over x^T.
"""

import numpy as np
import ml_dtypes

import concourse.bass as bass
import concourse.bacc as bacc
import concourse.tile as tile
import concourse.mybir as mybir
from concourse import masks
from concourse.bass_utils import run_bass_kernel_spmd

F32 = mybir.dt.float32
F32R = mybir.dt.float32r
BF16 = mybir.dt.bfloat16
AF = mybir.ActivationFunctionType
ALU = mybir.AluOpType
AX = mybir.AxisListType

B, S, D, H, FF = 4, 4096, 1024, 16, 4096
DV = D // H            # 64
P = 128
N_CORES = 8
S_LOC = 2048           # rows per core
T = S_LOC // P         # 16 s-tiles per core
KT = D // P            # 8 k-tiles over D
MT = FF // P           # 32 dff tiles
EPS = 1e-6
import os
NO_CC = bool(int(os.environ.get("BASS_NO_CC", "0")))
SQ = 4                 # ffn processes s in 4 quarters of 512 rows
TQ = T // SQ
NCH = S_LOC // 512     # 4 projection chunks of 512 cols


def build_program(affine_trivial=False):
    nc = bacc.Bacc("TRN2", target_bir_lowering=False, debug=False,
                   num_devices=N_CORES)

    # ---- DRAM I/O ----
    d = {}

    def din(name, shape, dtype=F32):
        d[name] = nc.dram_tensor(name, list(shape), dtype,
                                 kind="ExternalInput").ap()

    din("x_loc", [S_LOC, D], BF16)
    din("enc_loc", [S_LOC, D], BF16)
    din("xTb", [P, KT, S_LOC], BF16)
    din("encTb", [P, KT, S_LOC], BF16)
    din("wqk1p", [P, KT, 5 * H], BF16)
    din("wk2p", [P, KT, H], BF16)
    din("wq2r", [P, KT, H], BF16)
    din("c2b", [P, H]); din("bq2b", [P, H])
    for w in ["wv1p", "wo1p", "wv2p", "wo2p"]:
        din(w, [P, KT, D], BF16)
    din("wf1p", [MT, P, KT, P], BF16)
    din("wf2p", [P, MT, D], BF16)
    din("bq1c", [H, 1]); din("bq2c", [H, 1])
    din("bk1c", [H, 1]); din("bk2c", [H, 1])
    din("bv1h", [H, DV]); din("bv2h", [H, DV])
    din("bo1r", [H, D], BF16); din("bo2r", [H, D], BF16)
    din("bf1c", [P, MT]); din("bf2b", [P, D], F32R)
    for v in ["g1b", "be1b", "g2b", "be2b", "g3b", "be3b"]:
        din(v, [P, D])
    din("maskh", [H, D]); din("maskT", [P, KT, H])
    din("U16", [H, H]); din("Bsel", [64, H]); din("BselT", [H, 64])
    out_dram = nc.dram_tensor("out_loc", [S_LOC, D], F32,
                              kind="ExternalOutput").ap()

    with tile.TileContext(nc) as tc:
        def pool(name, bufs, side="left", space="SBUF"):
            return tc.alloc_tile_pool(name=name, bufs=bufs, side=side,
                                      space=space)

        # ============ long-lived pools ============
        cpool = pool("consts", 1)
        wf1_pool = pool("wf1", 4)
        ln_pool = pool("ln", 2, side="right")
        state_pool = pool("state", 1, side="right")
        apool = pool("attnaux", 1, side="right")
        sigq2_pool = pool("sigq2", 1, side="right")
        xk_pool = pool("xkeep", 1, side="right")
        row_pool = pool("rows", 5, side="right")
        sigq1_pool = pool("sigq1", 1, side="right")
        ma1_pool = pool("ma1", 1, side="right")
        sigk_pool = pool("sigk", 1, side="right")

        ident = cpool.tile([P, P], F32, name="ident")
        masks.make_identity(nc, ident[:])
        identr = cpool.tile([P, P], F32R, name="identr")
        nc.vector.tensor_copy(identr[:], ident[:])
        identb = cpool.tile([P, P], BF16, name="identb")
        nc.vector.tensor_copy(identb[:], ident[:])

        def load_const(key, dtype=F32, pl=None):
            pl = pl if pl is not None else cpool
            t_ = pl.tile([int(s) for s in d[key].shape], dtype,
                         name=f"c_{key}")
            nc.sync.dma_start(t_[:], d[key][:])
            return t_

        wqk1 = load_const("wqk1p", BF16)
        wk2 = load_const("wk2p", BF16)
        maskh = load_const("maskh")
        maskT = load_const("maskT")
        U16 = load_const("U16")
        Bsel = load_const("Bsel")
        BselT = load_const("BselT")
        bq1c = load_const("bq1c"); bq2c = load_const("bq2c")
        bk1c = load_const("bk1c"); bk2c = load_const("bk2c")
        bv1h = load_const("bv1h"); bv2h = load_const("bv2h")
        bo1r = load_const("bo1r", BF16); bo2r = load_const("bo2r", BF16)
        bf1c = load_const("bf1c")
        bf2br = load_const("bf2b", F32R)
        wq2r = load_const("wq2r", BF16)
        c2b = load_const("c2b"); bq2b = load_const("bq2b")
        eps = cpool.tile([P, 1], F32, name="epsc")
        nc.vector.memset(eps[:], EPS)

        dram_pool = pool("ccdram", 1, space="DRAM")
        cc1_in = dram_pool.tile([64, 65], F32, name="cc1_in")
        cc1_out = dram_pool.tile([64, 65], F32, name="cc1_out")
        cc2_in = dram_pool.tile([64, 65], F32, name="cc2_in")
        cc2_out = dram_pool.tile([64, 65], F32, name="cc2_out")
        out1d = dram_pool.tile([S_LOC, D], BF16, name="out1d")
        out2d = dram_pool.tile([S_LOC, D], BF16, name="out2d")

        sigq1 = sigq1_pool.tile([H, S_LOC], BF16, name="sigq1")
        sigq2 = sigq2_pool.tile([H, S_LOC], BF16, name="sigq2")
        sigk1 = sigk_pool.tile([P, T, H], BF16, name="sigk1")
        sigk2 = sigk_pool.tile([P, T, H], BF16, name="sigk2")

        def layernorm(res_ps, out_slice, g_sb, be_sb, lnp, tname,
                      rstd_out=None, nmr_out=None):
            """LN of psum tile res_ps [128,1024] -> out_slice (SBUF)."""
            st6 = lnp.tile([P, 2, 6], F32, tag="st6", name=f"st6_{tname}")
            nc.vector.bn_stats(st6[:, 0, :], res_ps[:, 0:512])
            nc.vector.bn_stats(st6[:, 1, :], res_ps[:, 512:1024])
            mv = lnp.tile([P, 2], F32, tag="mv", name=f"mv_{tname}")
            nc.vector.bn_aggr(mv[:], st6[:])
            std = lnp.tile([P, 1], F32, tag="std", name=f"std_{tname}")
            nc.scalar.activation(std[:], mv[:, 1:2], AF.Sqrt, bias=eps[:])
            if rstd_out is None:
                rstd = lnp.tile([P, 1], F32, tag="rstd", name=f"rstd_{tname}")
                rstd = rstd[:]
            else:
                rstd = rstd_out
            nc.vector.reciprocal(rstd, std[:])
            if nmr_out is None:
                nmr = lnp.tile([P, 1], F32, tag="nmr", name=f"nmr_{tname}")
                nmr = nmr[:]
            else:
                nmr = nmr_out
            nc.vector.scalar_tensor_tensor(nmr, mv[:, 0:1], -1.0, rstd,
                                           op0=ALU.mult, op1=ALU.mult)
            if affine_trivial:
                nc.scalar.activation(out_slice, res_ps[:], AF.Identity,
                                     bias=nmr, scale=rstd)
            else:
                xh = lnp.tile([P, 1024], F32, tag="ot", name=f"xh_{tname}")
                nc.scalar.activation(xh[:], res_ps[:], AF.Identity,
                                     bias=nmr, scale=rstd)
                nc.vector.scalar_tensor_tensor(out_slice, xh[:], 1.0, g_sb[:],
                                               op0=ALU.mult, op1=ALU.mult)
                nc.vector.tensor_add(out_slice, out_slice, be_sb[:])
            return rstd, nmr

        def attn_state(G_ps, sksum, tp2_psum, cc_in, nm):
            """G psum [16,1024] + sigma(k)-rowsum [16,1] -> kv[16,65]
            -> batch-slot select -> DMA."""
            gtmp = state_pool.tile([16, 1024], F32, tag="gtmp",
                                   name=f"gtmp_{nm}")
            nc.vector.tensor_mul(gtmp[:], G_ps[:, 0:1024], maskh[:])
            kvp = state_pool.tile([16, 65], F32, tag="kvp", name=f"kvp_{nm}")
            nc.vector.tensor_reduce(
                kvp[:, 0:64], gtmp[:].rearrange("p (c v) -> p v c", v=DV),
                axis=AX.X, op=ALU.add)
            nc.vector.tensor_copy(kvp[:, 64:65], sksum[:])
            kvsel_ps = tp2_psum.tile([64, 65], F32, tag="sel",
                                     name=f"kvselp_{nm}")
            nc.tensor.matmul(kvsel_ps[:], BselT[:], kvp[:],
                             start=True, stop=True)
            kvsel = state_pool.tile([64, 65], F32, tag="kvsel",
                                    name=f"kvsel_{nm}")
            nc.vector.tensor_copy(kvsel[:], kvsel_ps[:])
            nc.sync.dma_start(cc_in[:], kvsel[:])

        def state_to_M(cc_out, wo_sb, bvh, bor, spsum, ma_pool, nm):
            """AllReduce out -> own-batch kv -> cumsum -> M_aug [16,1024]."""
            kvred = state_pool.tile([64, 65], F32, tag="kvred",
                                    name=f"kvred_{nm}")
            nc.sync.dma_start(kvred[:], cc_out[:])
            kvmy_ps = spsum.tile([16, 65], F32, tag="sm", name=f"kvmyp_{nm}")
            nc.tensor.matmul(kvmy_ps[:], Bsel[:], kvred[:],
                             start=True, stop=True)
            kvmy = state_pool.tile([16, 65], F32, tag="kvmy",
                                   name=f"kvmy_{nm}")
            nc.vector.tensor_copy(kvmy[:], kvmy_ps[:])
            kv_bv = state_pool.tile([16, 64], F32, tag="kv_bv",
                                    name=f"kv_bv_{nm}")
            nc.vector.scalar_tensor_tensor(
                kv_bv[:], bvh[:], kvmy[:, 64:65], kvmy[:, 0:64],
                op0=ALU.mult, op1=ALU.add)
            scum_ps = spsum.tile([16, 64], F32, tag="sm", name=f"scump_{nm}")
            nc.tensor.matmul(scum_ps[:], U16[:], kv_bv[:],
                             start=True, stop=True)
            scum = state_pool.tile([16, 64], F32, tag="scumsb",
                                   name=f"scum_{nm}")
            nc.vector.tensor_copy(scum[:], scum_ps[:])
            scumT_ps = spsum.tile([64, 16], F32, tag="sm", name=f"scumTp_{nm}")
            nc.tensor.matmul(scumT_ps[:], scum[:], ident[:16, :16],
                             is_transpose=True)
            scumT2 = state_pool.tile([P, 16], F32, tag="scumT2",
                                     name=f"scumT2_{nm}")
            nc.vector.tensor_copy(scumT2[0:64, :], scumT_ps[:])
            nc.vector.tensor_copy(scumT2[64:P, :], scumT_ps[:])
            bdts = []
            for kt in range(KT):
                bdt = state_pool.tile([P, 16], BF16, tag=f"bdt{kt}",
                                      name=f"bdt_{nm}_{kt}")
                nc.vector.tensor_mul(bdt[:], scumT2[:], maskT[:, kt, :])
                bdts.append(bdt)
            Ma = ma_pool.tile([H, 1024], BF16, name=f"Ma_{nm}")
            for n2 in range(2):
                sl = slice(n2 * 512, (n2 + 1) * 512)
                M_ps = spsum.tile([16, 512], F32, tag="sm",
                                  name=f"M_{nm}_{n2}")
                for kt in range(KT):
                    nc.tensor.matmul(M_ps[:], bdts[kt][:], wo_sb[:, kt, sl],
                                     start=(kt == 0), stop=False)
                nc.tensor.matmul(M_ps[:], identb[:16, :16], bor[:, sl],
                                 start=False, stop=True)
                nc.vector.tensor_copy(Ma[:, sl], M_ps[:])
            return Ma

        def attn_side(xT_ap, x_row_ap, wproj, nheads_out, bqc, bkc, sigq_dst,
                      xT_pool_l, sigk_dst, wv_sb, cc_in, qk_psum, tp_psum,
                      ag_psum, nm, xq2_dst=None, xtc_eng=None, xk_keep=None):
            """One attention input side: projections from xT (bf16, DMA'd on
            the fly), sigmoids, A = sig(k)^T x, G = A @ wv, state -> cc."""
            sigkT = apool.tile([H, S_LOC], BF16, tag="sigkT",
                               name=f"sigkT_{nm}")
            xrows = []
            for t in range(T):
                if xk_keep is not None:
                    xr = xk_keep[:, t, :]
                else:
                    xrt = row_pool.tile([P, D], BF16, tag="xr",
                                        name=f"xr_{nm}{t}")
                    xr = xrt[:]
                nc.sync.dma_start(xr, x_row_ap[t * P:(t + 1) * P, :])
                xrows.append(xr)
            A_ps = ag_psum.tile([16, 1024], F32, tag="ag", name=f"A_{nm}")
            for n in range(NCH):
                csl = slice(n * 512, (n + 1) * 512)
                xTc = xT_pool_l.tile([P, KT, 512], BF16, tag="xTc",
                                     name=f"xTc_{nm}{n}")
                xtc_eng.dma_start(xTc[:], xT_ap[:, :, csl])
                qk_ps = qk_psum.tile([nheads_out, 512], F32, tag="qk",
                                     name=f"qk_{nm}{n}")
                for kt in range(KT):
                    nc.tensor.matmul(qk_ps[:], wproj[:, kt, :],
                                     xTc[:, kt, :],
                                     start=(kt == 0), stop=(kt == KT - 1))
                if nheads_out >= 4 * H:
                    nc.scalar.activation(sigq_dst[0:H, csl], qk_ps[0:H, :],
                                         AF.Sigmoid, bias=bqc[:])
                    nc.scalar.activation(sigkT[:, csl], qk_ps[2 * H:3 * H, :],
                                         AF.Sigmoid, bias=bkc[:])
                    if xq2_dst is not None:
                        nc.scalar.copy(xq2_dst[:, csl],
                                       qk_ps[4 * H:5 * H, :])
                else:
                    nc.scalar.activation(sigkT[:, csl], qk_ps[:], AF.Sigmoid,
                                         bias=bkc[:])
            # transpose sig(k) -> [s,16] tiles; A-matmuls (PE stays busy on
            # later qk chunks while ACT computes the earlier sigmoids)
            for t in range(T):
                skT_ps = tp_psum.tile([P, H], BF16, tag="tp",
                                      name=f"skT_{nm}{t}")
                nc.tensor.matmul(
                    skT_ps[:],
                    sigkT[:, t * P:(t + 1) * P],
                    identb[:16, :16], is_transpose=True)
                nc.vector.tensor_copy(sigk_dst[:, t, :], skT_ps[:])
                nc.tensor.matmul(A_ps[:, 0:512], sigk_dst[:, t, :],
                                 xrows[t][:, 0:512],
                                 start=(t == 0), stop=(t == T - 1))
                nc.tensor.matmul(A_ps[:, 512:1024], sigk_dst[:, t, :],
                                 xrows[t][:, 512:1024],
                                 start=(t == 0), stop=(t == T - 1))
            sksum = apool.tile([H, 1], F32, tag="sksum", name=f"sksum_{nm}")
            nc.vector.tensor_reduce(sksum[:], sigkT[:], axis=AX.X, op=ALU.add)
            A_sb = apool.tile([H, 1024], BF16, tag="A_sb", name=f"Asb_{nm}")
            nc.scalar.copy(A_sb[:], A_ps[:])
            AT = apool.tile([P, KT, H], BF16, tag="AT", name=f"AT_{nm}")
            for kt in range(KT):
                at_ps = tp_psum.tile([P, H], BF16, tag="tp",
                                     name=f"at_{nm}{kt}")
                nc.tensor.matmul(at_ps[:],
                                 A_sb[:, kt * P:(kt + 1) * P],
                                 identb[:16, :16], is_transpose=True)
                nc.vector.tensor_copy(AT[:, kt, :], at_ps[:])
            G_ps = ag_psum.tile([16, 1024], F32, tag="ag", name=f"G_{nm}")
            for kt in range(KT):
                for n2 in range(2):
                    sl = slice(n2 * 512, (n2 + 1) * 512)
                    nc.tensor.matmul(G_ps[:, sl], AT[:, kt, :],
                                     wv_sb[:, kt, sl],
                                     start=(kt == 0), stop=(kt == KT - 1))
            attn_state(G_ps, sksum, ag_psum, cc_in, nm)

        # ================= PHASE A1: x side =================
        xTc_pool = pool("xTc", 3)
        xk = xk_pool.tile([P, T, D], BF16, name="xk")
        wv1_pool = pool("wv1", 1)
        wv1 = wv1_pool.tile([P, KT, D], BF16, name="wv1")
        for kt in range(KT):
            nc.scalar.dma_start(wv1[:, kt, :], d["wv1p"][:, kt, :])

        qk_psum = pool("qk_psum", 2, space="PSUM")
        tp_psum = pool("tp_psum", 2, space="PSUM")
        ag_psum = pool("ag_psum", 1, space="PSUM")

        xq2T = sigq1_pool.tile([H, S_LOC], BF16, name="xq2T")
        attn_side(d["xTb"], d["x_loc"], wqk1, 5 * H, bq1c, bk1c, sigq1,
                  xTc_pool, sigk1, wv1, cc1_in, qk_psum, tp_psum, ag_psum,
                  "x", xq2_dst=xq2T, xtc_eng=nc.gpsimd, xk_keep=xk)
        if NO_CC:
            nc.sync.dma_start(cc1_out[:], cc1_in[:])
        else:
            nc.gpsimd.collective_compute(
                "AllReduce", ALU.add, replica_groups=[list(range(N_CORES))],
                ins=[cc1_in.opt()], outs=[cc1_out.opt()])
        wv1_pool.release()

        # ================= PHASE A2: enc side (overlaps AllReduce 1) ====
        wo1_pool = pool("wo1", 1)
        wo1 = wo1_pool.tile([P, KT, D], BF16, name="wo1")
        for kt in range(KT):
            nc.gpsimd.dma_start(wo1[:, kt, :], d["wo1p"][:, kt, :])
        wv2_pool = pool("wv2", 1)
        wv2 = wv2_pool.tile([P, KT, D], BF16, name="wv2")
        for kt in range(KT):
            nc.gpsimd.dma_start(wv2[:, kt, :], d["wv2p"][:, kt, :])

        attn_side(d["encTb"], d["enc_loc"], wk2, H, None, bk2c, None,
                  xTc_pool, sigk2, wv2, cc2_in, qk_psum, tp_psum, ag_psum,
                  "e", xtc_eng=nc.scalar)
        if NO_CC:
            nc.sync.dma_start(cc2_out[:], cc2_in[:])
        else:
            nc.gpsimd.collective_compute(
                "AllReduce", ALU.add, replica_groups=[list(range(N_CORES))],
                ins=[cc2_in.opt()], outs=[cc2_out.opt()])
        wv2_pool.release()

        # m1 state chain (waits AR1; AR1 landed during enc side)
        sps1 = pool("sps1", 1, space="PSUM")
        Ma1 = state_to_M(cc1_out, wo1, bv1h, bo1r, sps1, ma1_pool, "m1")
        sps1.release()
        wo1_pool.release()
        xTc_pool.release()
        ag_psum.release()
        tp_psum.release()
        qk_psum.release()
        sigk_pool.release()

        # ============ PHASE B: attn1 + LN1 + q2 (LN algebra) ============
        # q2 = sigmoid(rstd1*(P1^T sigq1 + (x@wq2eff)^T) + nmr1*c2 + cadd)
        # with P1 = Ma1 @ wq2eff; avoids materializing out1^T entirely.
        o2row_pool = pool("o2row", 4)
        o2T_pool = pool("o2T", 1)
        hT_pool = pool("hT", 1)
        q2aux_pool = pool("q2aux", 2)
        sq2all_pool = pool("sq2all", 1)
        sq2all = sq2all_pool.tile([P, T, H], F32, name="sq2all")
        if affine_trivial:
            gbe1_pool = g1b = be1b = None
        else:
            gbe1_pool = pool("gbe1", 1)
            g1b = load_const("g1b", pl=gbe1_pool)
            be1b = load_const("be1b", pl=gbe1_pool)

        a_psum = pool("a_psum", 2, side="right", space="PSUM")
        tpB = pool("tpB", 1, space="PSUM")
        q2ps = pool("q2ps", 1, space="PSUM")

        Ma1T = q2aux_pool.tile([P, KT, H], BF16, name="Ma1T")
        for kt in range(KT):
            mt_ps = tpB.tile([P, H], BF16, tag="tpb", name=f"ma1T_{kt}")
            nc.tensor.matmul(mt_ps[:],
                             Ma1[:, kt * P:(kt + 1) * P],
                             identb[:16, :16], is_transpose=True)
            nc.vector.tensor_copy(Ma1T[:, kt, :], mt_ps[:])
        P1_ps = q2ps.tile([H, H], F32, tag="q2", name="P1ps")
        for kt in range(KT):
            nc.tensor.matmul(P1_ps[:], Ma1T[:, kt, :], wq2r[:, kt, :],
                             start=(kt == 0), stop=(kt == KT - 1))
        P1 = q2aux_pool.tile([H, H], BF16, name="P1")
        nc.scalar.copy(P1[:], P1_ps[:])

        yq2sb = q2aux_pool.tile([H, S_LOC], F32R, name="yq2sb")
        rstdall = q2aux_pool.tile([P, T], F32, name="rstdall")
        nmrall = q2aux_pool.tile([P, T], F32, name="nmrall")
        bq2bz = q2aux_pool.tile([P, NCH, H], F32, name="bq2bz")
        zeroh = q2aux_pool.tile([P, H], F32, name="zeroh")
        nc.vector.memset(zeroh[:], 0.0)

        for n in range(NCH):
            csl = slice(n * 512, (n + 1) * 512)
            yq2_ps = q2ps.tile([H, 512], F32, tag="q2", name=f"yq2_{n}")
            nc.tensor.matmul(yq2_ps[:], P1[:], sigq1[:, csl],
                             start=True, stop=False)
            nc.tensor.matmul(yq2_ps[:], identb[:16, :16], xq2T[:, csl],
                             start=False, stop=True)
            nc.scalar.copy(yq2sb[:, csl], yq2_ps[:])
            for t4 in range(4):
                t = n * 4 + t4
                ap_ = a_psum.tile([P, D], F32, tag="a", name=f"a1_{t}")
                for n2 in range(2):
                    sl = slice(n2 * 512, (n2 + 1) * 512)
                    nc.tensor.matmul(ap_[:, sl],
                                     sigq1[:, t * P:(t + 1) * P],
                                     Ma1[:, sl], start=True, stop=False)
                    nc.tensor.matmul(ap_[:, sl], identb[:], xk[:, t, sl],
                                     start=False, stop=True)
                o1t = ln_pool.tile([P, D], BF16, tag="otb", name=f"o1t_{t}")
                layernorm(ap_, o1t[:], g1b, be1b, ln_pool, f"ln1_{t}",
                          rstd_out=rstdall[:, t:t + 1],
                          nmr_out=nmrall[:, t:t + 1])
                nc.gpsimd.dma_start(out1d[t * P:(t + 1) * P, :], o1t[:])
                if t4 == 3:
                    # bq2b + 0*rstd[last-of-chunk]: chunk-local gate so the
                    # sigmoid table flips once per chunk, not per tile
                    zc = q2aux_pool.tile([P, H], F32, tag="zch",
                                         name=f"zch_{n}")
                    nc.vector.scalar_tensor_tensor(
                        zc[:], zeroh[:], rstdall[:, t:t + 1], zeroh[:],
                        op0=ALU.mult, op1=ALU.add)
                    nc.vector.tensor_add(bq2bz[:, n, :], bq2b[:], zc[:])
        # q2 assembly tail
        for t in range(T):
            yq2r_ps = tpB.tile([P, H], F32, tag="tp", name=f"yq2r_{t}")
            nc.tensor.matmul(
                yq2r_ps[:],
                yq2sb[:, t * P:(t + 1) * P].bitcast(F32),
                ident[:16, :16], is_transpose=True)
            s2 = q2aux_pool.tile([P, H], F32, tag="s2", name=f"s2_{t}")
            nc.vector.scalar_tensor_tensor(s2[:], c2b[:],
                                           nmrall[:, t:t + 1],
                                           bq2bz[:, t // 4, :],
                                           op0=ALU.mult, op1=ALU.add)
            s1 = q2aux_pool.tile([P, H], F32, tag="s1", name=f"s1_{t}")
            nc.vector.scalar_tensor_tensor(s1[:], yq2r_ps[:],
                                           rstdall[:, t:t + 1], s2[:],
                                           op0=ALU.mult, op1=ALU.add)
            nc.scalar.activation(sq2all[:, t, :], s1[:], AF.Sigmoid)
            sq2T_ps = tpB.tile([H, P], F32, tag="tp2", name=f"sq2T_{t}")
            nc.tensor.matmul(sq2T_ps[:], sq2all[:, t, :], ident[:],
                             is_transpose=True)
            nc.vector.tensor_copy(sigq2[:, t * P:(t + 1) * P], sq2T_ps[:])
        q2ps.release()
        tpB.release()
        ma1_pool.release()
        sigq1_pool.release()
        row_pool.release()
        xk_pool.release()
        if gbe1_pool is not None:
            gbe1_pool.release()
        sq2all_pool.release()
        q2aux_pool.release()

        # ---- m2 state chain ----
        ma2_pool = pool("ma2", 1)
        if affine_trivial:
            gbe2_pool = g2b = be2b = None
        else:
            gbe2_pool = pool("gbe2", 1)
            g2b = load_const("g2b", pl=gbe2_pool)
            be2b = load_const("be2b", pl=gbe2_pool)
        o1row_pool = pool("o1row", 2)
        wo2_pool = pool("wo2", 1)
        wo2 = wo2_pool.tile([P, KT, D], BF16, name="wo2")
        for kt in range(KT):
            nc.sync.dma_start(wo2[:, kt, :], d["wo2p"][:, kt, :])
        sps2 = pool("sps2", 1, space="PSUM")
        Ma2 = state_to_M(cc2_out, wo2, bv2h, bo2r, sps2, ma2_pool, "m2")
        sps2.release()
        wo2_pool.release()

        wf2_pool = pool("wf2", 1)
        wf2 = wf2_pool.tile([P, MT, D], BF16, name="wf2")
        for m in range(MT):
            nc.gpsimd.dma_start(wf2[:, m, :], d["wf2p"][:, m, :])
        if affine_trivial:
            gbe3_pool = g3b = be3b = None
        else:
            gbe3_pool = pool("gbe3", 1)
            g3b = load_const("g3b", pl=gbe3_pool)
            be3b = load_const("be3b", pl=gbe3_pool)
        ln3_pool = pool("ln3", 2)
        h_psum = pool("h_psum", 3, space="PSUM")

        # ====== PHASE C+D: attn2+LN2 interleaved with FFN blocks ======
        def attn2_group(g):
            for t4 in range(TQ):
                t = g * TQ + t4
                ap2 = a_psum.tile([P, D], F32, tag="a", name=f"a2_{t}")
                o1r = o1row_pool.tile([P, D], BF16, tag="o1r",
                                      name=f"o1r_{t}")
                nc.sync.dma_start(o1r[:], out1d[t * P:(t + 1) * P, :])
                for n2 in range(2):
                    sl = slice(n2 * 512, (n2 + 1) * 512)
                    nc.tensor.matmul(ap2[:, sl],
                                     sigq2[:, t * P:(t + 1) * P],
                                     Ma2[:, sl], start=True, stop=False)
                    nc.tensor.matmul(ap2[:, sl], identb[:], o1r[:, sl],
                                     start=False, stop=True)
                o2t = ln_pool.tile([P, D], BF16, tag="otb", name=f"o2t_{t}")
                layernorm(ap2, o2t[:], g2b, be2b, ln_pool, f"ln2_{t}")
                nc.gpsimd.dma_start(out2d[t * P:(t + 1) * P, :], o2t[:])

        def ffn_block(sq):
            """FFN for s-quarter sq; o2T via DMA-transpose XBAR."""
            o2T = o2T_pool.tile([P, KT, TQ * P], BF16, tag="o2T",
                                name=f"o2T_{sq}")
            nc.sync.dma_start_transpose(
                o2T[:], out2d[sq * TQ * P:(sq + 1) * TQ * P, :])
            o2rows = []
            for t4 in range(TQ):
                t = sq * TQ + t4
                o2r = o2row_pool.tile([P, D], BF16, tag="o2r",
                                      name=f"o2r_{t}")
                nc.sync.dma_start(o2r[:], out2d[t * P:(t + 1) * P, :])
                o2rows.append(o2r)
            hT = hT_pool.tile([P, MT, TQ * P], BF16, tag="hT",
                              name=f"hT_{sq}")
            for m in range(MT):
                wf1m = wf1_pool.tile([P, KT, P], BF16, tag="wf1m",
                                     name=f"wf1_{sq}_{m}")
                nc.gpsimd.dma_start(wf1m[:], d["wf1p"][m])
                hp = h_psum.tile([P, TQ * P], F32, tag="hp",
                                 name=f"hp_{sq}_{m}")
                for kt in range(KT):
                    nc.tensor.matmul(hp[:], wf1m[:, kt, :], o2T[:, kt, :],
                                     start=(kt == 0), stop=(kt == KT - 1))
                nc.scalar.activation(hT[:, m, :], hp[:], AF.Relu,
                                     bias=bf1c[:, m:m + 1])
            for t4 in range(TQ):
                t = sq * TQ + t4
                o3 = ln3_pool.tile([P, D], F32, tag="o3f", name=f"o3f_{t}")
                st6 = ln3_pool.tile([P, 2, 6], F32, tag="st6",
                                    name=f"st6f_{t}")
                chunks = []
                for n2 in range(2):
                    sl = slice(n2 * 512, (n2 + 1) * 512)
                    op3 = h_psum.tile([P, 512], F32, tag="hp",
                                      name=f"o3c_{t}_{n2}")
                    for m in range(MT):
                        nc.tensor.matmul(op3[:],
                                         hT[:, m, t4 * P:(t4 + 1) * P],
                                         wf2[:, m, sl],
                                         start=(m == 0), stop=False)
                    nc.tensor.matmul(op3[:], identb[:], o2rows[t4][:, sl],
                                     start=False, stop=affine_trivial)
                    if not affine_trivial:
                        nc.tensor.matmul(op3[:], identr[:], bf2br[:, sl],
                                         start=False, stop=True)
                    nc.vector.bn_stats(st6[:, n2, :], op3[:])
                    chunks.append(op3)
                mv = ln3_pool.tile([P, 2], F32, tag="mv", name=f"mvf_{t}")
                nc.vector.bn_aggr(mv[:], st6[:])
                std = ln3_pool.tile([P, 1], F32, tag="std", name=f"stdf_{t}")
                nc.scalar.activation(std[:], mv[:, 1:2], AF.Sqrt,
                                     bias=eps[:])
                rstd = ln3_pool.tile([P, 1], F32, tag="rstd",
                                     name=f"rstdf_{t}")
                nc.vector.reciprocal(rstd[:], std[:])
                nmr = ln3_pool.tile([P, 1], F32, tag="nmr", name=f"nmrf_{t}")
                nc.vector.scalar_tensor_tensor(nmr[:], mv[:, 0:1], -1.0,
                                               rstd[:], op0=ALU.mult,
                                               op1=ALU.mult)
                for n2 in range(2):
                    sl = slice(n2 * 512, (n2 + 1) * 512)
                    nc.scalar.activation(o3[:, sl], chunks[n2][:],
                                         AF.Identity, bias=nmr[:],
                                         scale=rstd[:])
                if not affine_trivial:
                    nc.vector.scalar_tensor_tensor(o3[:], o3[:], 1.0,
                                                   g3b[:], op0=ALU.mult,
                                                   op1=ALU.mult)
                    nc.vector.tensor_add(o3[:], o3[:], be3b[:])
                nc.scalar.dma_start(out_dram[t * P:(t + 1) * P, :], o3[:])

        attn2_group(0)
        attn2_group(1)
        ffn_block(0)
        attn2_group(2)
        ffn_block(1)
        attn2_group(3)
        ffn_block(2)
        ffn_block(3)

        a_psum.release()
        rel = [h_psum, ln3_pool]
        if gbe3_pool is not None:
            rel.append(gbe3_pool)
        rel.append(wf2_pool)
        rel.append(o1row_pool)
        if gbe2_pool is not None:
            rel.append(gbe2_pool)
        rel += [ma2_pool, hT_pool, o2T_pool, o2row_pool, wf1_pool,
                cpool, dram_pool, sigq2_pool, apool, state_pool, ln_pool]
        for p_ in rel:
            p_.release()

    nc.compile()
    return nc


_NC_CACHE = {}


def _get_nc(affine_trivial):
    if affine_trivial not in _NC_CACHE:
        _NC_CACHE[affine_trivial] = build_program(affine_trivial)
    return _NC_CACHE[affine_trivial]


def _affine_trivial(inputs):
    for g in ("g1", "g2", "g3"):
        if not np.all(np.asarray(inputs[g]) == 1.0):
            return False
    for b in ("be1", "be2", "be3", "bf2"):
        if not np.all(np.asarray(inputs[b]) == 0.0):
            return False
    return True


def _prep_inputs(inputs):
    f32 = lambda a: np.ascontiguousarray(np.asarray(a, dtype=np.float32))
    x = f32(inputs["x"])
    enc = f32(inputs["enc"])

    def pack_w(w):  # [D, n] -> [P, KT, n]
        w = f32(w)
        return np.ascontiguousarray(w.reshape(KT, P, -1).transpose(1, 0, 2))

    zpad = np.zeros((D, H), np.float32)
    wq2eff = f32(inputs["g1"]).reshape(D, 1) * f32(inputs["wq2"])
    wqk1 = np.concatenate([f32(inputs["wq1"]), zpad, f32(inputs["wk1"]),
                           zpad, wq2eff], axis=1)
    cadd = (f32(inputs["be1"]) @ f32(inputs["wq2"])
            + f32(inputs["bq2"]).reshape(-1))
    shared = {
        "wqk1p": pack_w(wqk1).astype(ml_dtypes.bfloat16),
        "wk2p": pack_w(inputs["wk2"]).astype(ml_dtypes.bfloat16),
        "wq2r": pack_w(wq2eff).astype(ml_dtypes.bfloat16),
        "wv1p": pack_w(inputs["wv1"]).astype(ml_dtypes.bfloat16),
        "wo1p": pack_w(inputs["wo1"]).astype(ml_dtypes.bfloat16),
        "wv2p": pack_w(inputs["wv2"]).astype(ml_dtypes.bfloat16),
        "wo2p": pack_w(inputs["wo2"]).astype(ml_dtypes.bfloat16),
    }
    shared["c2b"] = np.ascontiguousarray(
        np.broadcast_to(wq2eff.sum(0)[None, :], (P, H)))
    shared["bq2b"] = np.ascontiguousarray(
        np.broadcast_to(cadd[None, :], (P, H)))
    wf1 = f32(inputs["wf1"])  # [D, FF]
    wf1p = wf1.reshape(KT, P, MT, P).transpose(2, 1, 0, 3)
    shared["wf1p"] = np.ascontiguousarray(wf1p.astype(ml_dtypes.bfloat16))
    wf2 = f32(inputs["wf2"])  # [FF, D]
    shared["wf2p"] = np.ascontiguousarray(
        wf2.reshape(MT, P, D).transpose(1, 0, 2).astype(ml_dtypes.bfloat16))

    def bcast(v):
        v = f32(v).reshape(-1)
        return np.ascontiguousarray(np.broadcast_to(v[None, :], (P, v.size)))

    shared["bq1c"] = f32(inputs["bq1"]).reshape(H, 1)
    shared["bq2c"] = f32(inputs["bq2"]).reshape(H, 1)
    shared["bk1c"] = f32(inputs["bk1"]).reshape(H, 1)
    shared["bk2c"] = f32(inputs["bk2"]).reshape(H, 1)
    shared["bv1h"] = f32(inputs["bv1"]).reshape(H, DV)
    shared["bv2h"] = f32(inputs["bv2"]).reshape(H, DV)
    shared["bo1r"] = np.ascontiguousarray(np.broadcast_to(
        f32(inputs["bo1"])[None, :], (H, D))).astype(ml_dtypes.bfloat16)
    shared["bo2r"] = np.ascontiguousarray(np.broadcast_to(
        f32(inputs["bo2"])[None, :], (H, D))).astype(ml_dtypes.bfloat16)
    shared["bf1c"] = np.ascontiguousarray(f32(inputs["bf1"]).reshape(MT, P).T)
    shared["bf2b"] = bcast(inputs["bf2"])
    for k_src, k_dst in [("g1", "g1b"), ("be1", "be1b"), ("g2", "g2b"),
                         ("be2", "be2b"), ("g3", "g3b"), ("be3", "be3b")]:
        shared[k_dst] = bcast(inputs[k_src])

    hh = np.arange(H)
    jj = np.arange(D)
    shared["maskh"] = (jj[None, :] // DV == hh[:, None]).astype(np.float32)
    pp = np.arange(P)
    kk = np.arange(KT)
    shared["maskT"] = ((kk[None, :, None] * P + pp[:, None, None]) // DV
                       == hh[None, None, :]).astype(np.float32)
    shared["U16"] = (hh[:, None] <= hh[None, :]).astype(np.float32)

    def packT(rows):  # [S_LOC, D] f32 -> [P, KT, S_LOC] bf16
        xt = rows.T.reshape(KT, P, S_LOC).transpose(1, 0, 2)
        return np.ascontiguousarray(xt.astype(ml_dtypes.bfloat16))

    in_maps = []
    for c in range(N_CORES):
        b, half = c // 2, c % 2
        s0 = half * S_LOC
        m = dict(shared)
        xl = np.ascontiguousarray(x[b, s0:s0 + S_LOC, :])
        el = np.ascontiguousarray(enc[b, s0:s0 + S_LOC, :])
        m["x_loc"] = xl.astype(ml_dtypes.bfloat16)
        m["enc_loc"] = el.astype(ml_dtypes.bfloat16)
        m["xTb"] = packT(xl)
        m["encTb"] = packT(el)
        bsel = (np.arange(64)[:, None] == 16 * b + hh[None, :]).astype(
            np.float32)
        m["Bsel"] = bsel
        m["BselT"] = np.ascontiguousarray(bsel.T)
        in_maps.append(m)
    return in_maps


def run_on_hw(inputs, **kwargs):
    nc = _get_nc(_affine_trivial(inputs))
    in_maps = _prep_inputs(inputs)
    return run_bass_kernel_spmd(nc, in_maps, list(range(N_CORES)), **kwargs)


def kernel(**inputs):
    r = run_on_hw(inputs)
    out = np.empty((B, S, D), dtype=np.float32)
    for c in range(N_CORES):
        b, half = c // 2, c % 2
        out[b, half * S_LOC:(half + 1) * S_LOC, :] = r.results[c]["out_loc"]
    return (out, np.zeros_like(out), np.zeros_like(out))
